# revision 31
# baseline (speedup 1.0000x reference)
"""MixLinear GEMM kernel for Trainium2 (8 NeuronCores, column-parallel).

Reference computation:
    inputs = x.reshape(-1, 4096)
    act_outliers = inputs[:, ind]
    inputs_z = inputs with ind-columns zeroed
    x_scale = clamp(rowmax(|inputs_z|)/127, 1e-8)
    q_x = round(inputs_z / x_scale)
    y = (q_x @ q_weight.T) * x_scale * scale_col + act_outliers @ weight_cache.T + bias

v3 device formulation (this file):
  * Host packs W[k, o] = q_weight[o, k] * scale_col[o] in fp16, k-tiled,
    PLUS two extra k-tiles holding weight_cache^T (the outlier weights).
  * Host supplies x in [M, K] (for the row absmax) and transposed [K, M]
    (so q^T is produced directly in matmul layout), plus the gathered
    outlier rows xT[ind, :] for the exact outlier path.
  * q is produced OFFSET BY 1536 and the GEMM consumes the offset value:
    one DVE multiply t = xT*recip (fp16) and ONE ACT pass
    q1536 = t*mask_k + 1536 -> fp16. fp16 spacing is 1.0 in [1024, 2048),
    so RNE rounds to an exact integer (q+1536 in [1409,1663] is exact);
    masked (outlier) rows give exactly 1536 (= q 0). The offset is removed
    in the epilogue via a per-column correction
    colcorr[o] = -1536 * colsum(W[o]) (host, fp32).
  * Outlier k-tiles of q^T are xo*recip in full precision (no rounding,
    no offset), so (qo @ wc) * xs = act_outliers @ weight_cache.T exactly
    as in the reference (the baseline only approximated this).
  * Epilogue: ps += colcorr (DVE, in PSUM); y = ps*xs + bias -> fp16.

Engine placement per rep: DVE does the mask multiplies, abs-max reduces,
scale finals, the xT*recip multiplies and the epilogues; ACT does the
single magic-round pass and issues y stores; Pool ONLY issues the weight
DMAs (a dma_start behind queued compute would delay the prefetch by the
whole compute chain); SP issues x/xT/xo/recip DMAs; PE runs 34 k-tiles
x 12 output tiles of matmuls with all 8 PSUM banks in flight.
The recip row broadcast [128, M] is built via a small DRAM round-trip
(stage transposed, re-read partition-broadcast). DMA issues are batched
(3D access patterns) because each dma_start costs ~0.6-1.5us of
sequencer descriptor generation.

Emission is software-pipelined and interleaved:
  phase1_absmax(r+1) | phase2(r, c0) | phase1_quant(r+1) | phase2(r, c1)
  | phase2(r, c2)
"""

import sys

import numpy as np

sys.path.insert(0, "/opt/trn_rl_repo")

import concourse.bass as bass  # noqa: E402
import concourse.mybir as mybir  # noqa: E402
import concourse.tile as tile  # noqa: E402
from concourse import bacc  # noqa: E402

N_CORES = 8
M = 512  # 8*64 rows
K = 4096  # in_features
OUT = 11008  # out_features
OSH = OUT // N_CORES  # 1376 per-core shard
FP = 256  # outlier columns
KT = K // 128  # 32 k-tiles
KO = 2  # outlier k-tiles
KTT = KT + KO  # 34 total k-tiles
MT = M // 128  # 4 m-tiles
MAGIC = 1536.0  # fp16 spacing is 1.0 in [1024, 2048): rounds q+1536 to int
OC = 459  # padded o-chunk width (fits one PSUM bank: 459*4B <= 2KB)
OCS = [459, 459, 458]  # actual chunk widths (sum = OSH)
OFF = [0, 459, 918]
NCH = 3  # chunks
XH = 2048  # x streamed in half-rows for the absmax
KG = 4  # k-tiles per xT load group

f32 = mybir.dt.float32
f16 = mybir.dt.float16
bf16 = mybir.dt.bfloat16
Alu = mybir.AluOpType
Act = mybir.ActivationFunctionType


def build_program(nrep=1):
    """Build the kernel program. nrep>1 emits the whole body nrep times
    (same inputs, same outputs) - used only to measure steady-state HW time
    as (t(nrep) - t(1)) / (nrep - 1)."""
    nc = bacc.Bacc(
        "TRN2", target_bir_lowering=False, debug=False, num_devices=N_CORES
    )

    x_d = nc.dram_tensor("x_in", [M, K], f16, kind="ExternalInput").ap()
    xt_d = nc.dram_tensor("xt_in", [K, M], f16, kind="ExternalInput").ap()
    xo_d = nc.dram_tensor("xo_in", [FP, M], f16, kind="ExternalInput").ap()
    w_d = nc.dram_tensor("w_in", [NCH, 128, KTT * OC], f16, kind="ExternalInput").ap()
    mask_d = nc.dram_tensor("mask_in", [1, K], f16, kind="ExternalInput").ap()
    maskk_d = nc.dram_tensor("maskk_in", [128, KT], f32, kind="ExternalInput").ap()
    bias_d = nc.dram_tensor("bias_in", [1, OSH], bf16, kind="ExternalInput").ap()
    cc_d = nc.dram_tensor("cc_in", [1, OSH], f32, kind="ExternalInput").ap()
    y_d = nc.dram_tensor("y_out", [M, OSH], f16, kind="ExternalOutput").ap()
    # fp16 recip staging for the row-broadcast (4 slots, rep%4-indexed),
    # stored transposed: rs[slot, m] = recip[m]
    rs_d = nc.dram_tensor("rs", [4, M], f16, kind="Internal").ap()

    with tile.TileContext(nc) as tc:
        with (
            tc.tile_pool(name="persist", bufs=1) as persist,
            tc.tile_pool(name="xpool", bufs=2) as xpool,
            tc.tile_pool(name="xzpool", bufs=2) as xzpool,
            tc.tile_pool(name="xtpool", bufs=2) as xtpool,
            tc.tile_pool(name="tpool", bufs=2) as tpool,
            tc.tile_pool(name="rbpool", bufs=1) as rbpool,
            tc.tile_pool(name="wtpool", bufs=3) as wtpool,
            tc.tile_pool(name="ypool", bufs=2) as ypool,
            tc.tile_pool(name="psmain", bufs=8, space="PSUM") as psmain,
        ):
            # ---------- persistent tiles ----------
            # q^T [k-part, kk, m] incl. 2 outlier k-tiles, per rep parity
            q_sets = [
                persist.tile([128, KTT, M], f16, tag=f"qT{par}", name=f"qT{par}")
                for par in range(2)
            ]
            mask_bc = persist.tile([128, K], f16)  # ind-mask broadcast (m-major)
            maskk = persist.tile([128, KT], f32)  # ind-mask, k-major per-partition
            bias_bc = persist.tile([128, OSH], bf16)
            cc_bc = persist.tile([128, OSH], f32)  # -384*colsum(W) broadcast
            am_parts = persist.tile([128, MT * 4], f32)
            am_all = persist.tile([128, MT], f32)
            xs_all = persist.tile([128, 4 * MT], f32)  # rep%4-indexed
            recip_all = persist.tile([128, 4 * MT], f32)
            recip16 = persist.tile([128, 4 * MT], f16)

            # ---------- setup ----------
            nc.gpsimd.dma_start(
                out=mask_bc,
                in_=bass.AP(mask_d.tensor, mask_d.offset, [[0, 128], [1, K]]),
            )
            nc.gpsimd.dma_start(out=maskk, in_=maskk_d)
            nc.gpsimd.dma_start(
                out=bias_bc,
                in_=bass.AP(bias_d.tensor, bias_d.offset, [[0, 128], [1, OSH]]),
            )
            nc.gpsimd.dma_start(
                out=cc_bc,
                in_=bass.AP(cc_d.tensor, cc_d.offset, [[0, 128], [1, OSH]]),
            )

            recip_bcs = {}

            def phase1_absmax(rep):
                """x loads, masked absmax (Pool mult + DVE reduce), scale
                finals (DVE), recip row-broadcast round-trip (SP)."""
                pq = rep % 4
                for mt in range(MT):
                    ms = slice(mt * 128, (mt + 1) * 128)
                    xhs = []
                    for h in range(2):
                        x_h = xpool.tile(
                            [128, XH], f16, tag="x", name=f"x_{rep}_{mt}_{h}"
                        )
                        nc.sync.dma_start(
                            out=x_h, in_=x_d[ms, h * XH : (h + 1) * XH]
                        )
                        xhs.append(x_h)
                    for h in range(2):
                        for qh in range(2):
                            xz = xzpool.tile(
                                [128, XH // 2],
                                f16,
                                tag="xz",
                                name=f"xz_{rep}_{mt}_{h}_{qh}",
                            )
                            lo = qh * (XH // 2)
                            nc.vector.tensor_tensor(
                                out=xz,
                                in0=xhs[h][:, lo : lo + XH // 2],
                                in1=mask_bc[
                                    :,
                                    h * XH + lo : h * XH + lo + XH // 2,
                                ],
                                op=Alu.mult,
                            )
                            pcol = mt * 4 + h * 2 + qh
                            nc.vector.tensor_reduce(
                                out=am_parts[:, pcol : pcol + 1],
                                in_=xz,
                                axis=mybir.AxisListType.X,
                                op=Alu.max,
                                apply_absolute_value=True,
                            )
                    nc.vector.tensor_reduce(
                        out=am_all[:, mt : mt + 1],
                        in_=am_parts[:, mt * 4 : mt * 4 + 4],
                        axis=mybir.AxisListType.X,
                        op=Alu.max,
                        apply_absolute_value=False,
                    )
                    pc = pq * MT + mt
                    # xs = max(absmax/127, 1e-8); recip = 1/xs
                    nc.vector.tensor_scalar(
                        xs_all[:, pc : pc + 1],
                        am_all[:, mt : mt + 1],
                        1.0 / 127.0,
                        1e-8,
                        Alu.mult,
                        Alu.max,
                    )
                    nc.vector.reciprocal(
                        out=recip_all[:, pc : pc + 1], in_=xs_all[:, pc : pc + 1]
                    )
                sl4 = slice(pq * MT, (pq + 1) * MT)
                nc.vector.tensor_scalar(
                    recip16[:, sl4], recip_all[:, sl4], 1.0, None, Alu.mult
                )
                # stage fp16 recip to DRAM slot pq in m-order (scatter write:
                # rs[pq, mt*128+p] = recip16[p, pq*MT+mt]), re-read broadcast
                nc.sync.dma_start(
                    out=bass.AP(
                        rs_d.tensor, rs_d.offset + pq * M, [[1, 128], [128, MT]]
                    ),
                    in_=recip16[:, sl4],
                )
                rb = rbpool.tile([128, M], f16, tag="rb", name=f"rb_{rep}")
                nc.sync.dma_start(
                    out=rb,
                    in_=bass.AP(
                        rs_d.tensor, rs_d.offset + pq * M, [[0, 128], [1, M]]
                    ),
                )
                recip_bcs[rep] = rb

            def phase1_quant(rep):
                """xT/xo loads (SP), per-k-tile t = xT*recip (DVE, fp16),
                single magic pass q384 = t*mask_k + 384 -> bf16 (ACT), and
                full-precision outlier tiles qo = xo*recip -> bf16 (DVE)."""
                par = rep % 2
                q_t = q_sets[par]
                rb = recip_bcs.pop(rep)
                for g in range(K // (KG * 128)):
                    xt = xtpool.tile(
                        [128, KG, M], f16, tag="xt", name=f"xt_{rep}_{g}"
                    )
                    nc.sync.dma_start(
                        out=xt,
                        in_=bass.AP(
                            xt_d.tensor,
                            xt_d.offset + g * KG * 128 * M,
                            [[M, 128], [128 * M, KG], [1, M]],
                        ),
                    )
                    for j in range(KG):
                        kk = g * KG + j
                        t16 = tpool.tile(
                            [128, M], f16, tag="t", name=f"t_{rep}_{kk}"
                        )
                        nc.vector.tensor_tensor(
                            out=t16, in0=xt[:, j, :], in1=rb, op=Alu.mult
                        )
                        nc.scalar.activation(
                            out=q_t[:, kk, :],
                            in_=t16,
                            func=Act.Copy,
                            bias=MAGIC,
                            scale=maskk[:, kk : kk + 1],
                        )
                # outlier k-tiles: full-precision xo*recip (no offset/mask)
                xo = xtpool.tile([128, KO, M], f16, tag="xt", name=f"xo_{rep}")
                nc.sync.dma_start(
                    out=xo,
                    in_=bass.AP(
                        xo_d.tensor,
                        xo_d.offset,
                        [[M, 128], [128 * M, KO], [1, M]],
                    ),
                )
                for j in range(KO):
                    nc.vector.tensor_tensor(
                        out=q_t[:, KT + j, :],
                        in0=xo[:, j, :],
                        in1=rb,
                        op=Alu.mult,
                    )

            def load_w(rep, c):
                wt = wtpool.tile(
                    [128, KTT, OC], f16, tag="wt", name=f"wt_{rep}_{c}"
                )
                nc.gpsimd.dma_start(out=wt, in_=w_d[c])
                return wt

            wt_cur = {}
            wt_next = {}

            def phase2_chunk(rep, c, prefetch_next):
                par = rep % 2
                pq = rep % 4
                q_t = q_sets[par]
                nonlocal wt_cur, wt_next
                wt = wt_cur[c]
                o0 = OFF[c]
                cw = OCS[c]
                ysb = ypool.tile(
                    [128, MT, OC], f16, tag="ysb", name=f"ysb_{rep}_{c}"
                )
                for mt in range(MT):
                    pc = pq * MT + mt
                    ps = psmain.tile(
                        [128, OC], f32, tag="ps", name=f"ps_{rep}_{c}_{mt}"
                    )
                    for kk in range(KTT):
                        nc.tensor.matmul(
                            ps,
                            lhsT=q_t[:, kk, mt * 128 : (mt + 1) * 128],
                            rhs=wt[:, kk, :],
                            start=(kk == 0),
                            stop=(kk == KTT - 1),
                        )
                    # remove the 384-offset: ps += -384*colsum(W)
                    nc.vector.tensor_tensor(
                        out=ps[:, :cw],
                        in0=ps[:, :cw],
                        in1=cc_bc[:, o0 : o0 + cw],
                        op=Alu.add,
                    )
                    # y = ps * xs + bias
                    nc.vector.scalar_tensor_tensor(
                        out=ysb[:, mt, :cw],
                        in0=ps[:, :cw],
                        scalar=xs_all[:, pc : pc + 1],
                        in1=bias_bc[:, o0 : o0 + cw],
                        op0=Alu.mult,
                        op1=Alu.add,
                    )
                # one 3D store for all 4 m-tiles of this chunk
                nc.scalar.dma_start(
                    out=bass.AP(
                        y_d.tensor,
                        y_d.offset + o0,
                        [[OSH, 128], [128 * OSH, MT], [1, cw]],
                    ),
                    in_=ysb[:, :, :cw],
                )
                if prefetch_next:
                    wt_next[c] = load_w(rep + 1, c)
                if c == NCH - 1 and prefetch_next:
                    wt_cur = wt_next
                    wt_next = {}

            # software-pipelined, chunk-interleaved emission
            wt_cur = {cc: load_w(0, cc) for cc in range(NCH)}
            phase1_absmax(0)
            phase1_quant(0)
            for rep in range(nrep):
                more = rep + 1 < nrep
                if more:
                    phase1_absmax(rep + 1)
                phase2_chunk(rep, 0, prefetch_next=more)
                if more:
                    phase1_quant(rep + 1)
                phase2_chunk(rep, 1, prefetch_next=more)
                phase2_chunk(rep, 2, prefetch_next=more)

    nc.compile()
    return nc


_NC_CACHE = None


def get_program():
    global _NC_CACHE
    if _NC_CACHE is None:
        _NC_CACHE = build_program()
    return _NC_CACHE


def make_in_maps(x, q_weight, scale_col, weight_cache, ind, bias):
    x2 = np.ascontiguousarray(
        np.asarray(x, dtype=np.float32).reshape(M, K).astype(np.float16)
    )
    xt = np.ascontiguousarray(x2.T)
    q_weight = np.asarray(q_weight, dtype=np.int32)
    scale_col = np.asarray(scale_col, dtype=np.float32).reshape(OUT)
    weight_cache = np.asarray(weight_cache, dtype=np.float32)
    ind_np = np.asarray(ind, dtype=np.int32).reshape(FP)
    bias_np = np.asarray(bias, dtype=np.float32).reshape(OUT)

    import ml_dtypes

    mask = np.ones(K, dtype=np.float32)
    mask[ind_np] = 0.0
    mask_bf = mask.astype(np.float16).reshape(1, K)
    # k-major per-partition mask: maskk[p, kk] = mask[kk*128 + p]
    maskk = np.ascontiguousarray(mask.reshape(KT, 128).T.astype(np.float32))
    # gathered outlier activation rows (layout op only)
    xo = np.ascontiguousarray(xt[ind_np, :])  # [FP, M]

    # combined weight: W[k, o] = q_weight[o, k] * scale_col[o]; outlier
    # rows stay as-is (masked q contributes 0 there, like the reference's
    # inputs_z); weight_cache goes in two extra k-tiles.
    wf = q_weight.astype(np.float32) * scale_col.reshape(OUT, 1)  # [OUT, K]
    wc16 = np.ascontiguousarray(wf.T).astype(np.float16)  # [K, OUT]
    wo16 = np.ascontiguousarray(weight_cache.T).astype(np.float16)  # [FP, OUT]
    # per-column correction: -384 * colsum of the bf16 main weights (fp32)
    colcorr = (-MAGIC * wc16.astype(np.float64).sum(0)).astype(np.float32)

    in_maps = []
    for core in range(N_CORES):
        sl = slice(core * OSH, (core + 1) * OSH)
        shard = wc16[:, sl].reshape(KT, 128, OSH)  # [kk, p, o]
        oshard = wo16[:, sl].reshape(KO, 128, OSH)
        # pack: [chunk, partition(k%128), kk(incl outlier tiles), o-in-chunk]
        wpack = np.zeros((NCH, 128, KTT, OC), dtype=wc16.dtype)
        for c in range(NCH):
            osl = slice(OFF[c], OFF[c] + OCS[c])
            wpack[c, :, :KT, : OCS[c]] = shard[:, :, osl].transpose(1, 0, 2)
            wpack[c, :, KT:, : OCS[c]] = oshard[:, :, osl].transpose(1, 0, 2)
        wpack = np.ascontiguousarray(wpack).reshape(NCH, 128, KTT * OC)
        in_maps.append(
            {
                "x_in": x2,
                "xt_in": xt,
                "xo_in": xo,
                "w_in": wpack,
                "mask_in": mask_bf,
                "maskk_in": maskk,
                "bias_in": np.ascontiguousarray(
                    bias_np[sl].astype(ml_dtypes.bfloat16).reshape(1, OSH)
                ),
                "cc_in": np.ascontiguousarray(colcorr[sl].reshape(1, OSH)),
            }
        )
    return in_maps


def kernel(x, q_weight, scale_col, weight_cache, ind, bias):
    from concourse.bass_utils import run_bass_kernel_spmd

    nc = get_program()
    in_maps = make_in_maps(x, q_weight, scale_col, weight_cache, ind, bias)
    res = run_bass_kernel_spmd(nc, in_maps, core_ids=list(range(N_CORES)))
    shards = [res.results[c]["y_out"] for c in range(N_CORES)]
    y = np.concatenate(shards, axis=1)
    return y.reshape(8, 64, OUT).astype(np.float32)
